# revision 17
# baseline (speedup 1.0000x reference)
"""Trainium2 Bass kernel for nn_AttentionBlock (GroupNorm + 1x1-conv attention).

Contract: kernel(**inputs) takes FULL unsharded inputs (numpy, shapes as in
setup_inputs) and returns the FULL output. Internally shards batch (32) over
8 NeuronCores (4 batch elements per core), params replicated.

v2: all five matmul families run in fp8-e4m3 DoubleRow mode (2 k-tiles of
128 per instruction, 0.5 cycles/row), with drain work balanced across the
DVE / Activation / Pool engines so no single engine is the wall.

Scaling scheme (all folded, zero extra ops):
  host: wq,wk,wv (and their biases) pre-scaled by 8 before the fp8 cast so
        weight entries sit in fp8's normal range; wo cast unscaled.
  q2 = 8q, k2 = 8k  ->  S' = 64 S; exp scale folds 1/64 into c^-0.5 and an
        extra exp bias of -ln(16) keeps P in fp8 range (max ~17 << 240).
  v' = 8v and P' = P/16 cancel against a ones-column value of 8 in the O
        matmul: O-drain x (1/Zcol) restores exactly O_true.

Math per batch element (faithful to the reference's raw channels-last
reshape): with q,k,v the (hw=1024, c=512) projection outputs, the raw
reshape to (c, hw) produces matrices whose row r is the concat of pixel
rows 2r and 2r+1.  We compute
    S^T = K2^T Q2    (contraction over the 512 "pixel-pair" axis)
    P^T = exp(S^T / sqrt(c)) / 16     (no max-subtraction; scores are O(1))
    Z   = colsum(P') via ones-matmul, 1/Z applied in the O-drain
    O^T = (P^T as lhsT) @ V2^T        -> raw-reshape layout
then un-reshape via an even/odd interleave copy and apply the final conv +
residual (residual added in PSUM via an identity matmul for most tiles).
GroupNorm rsqrt runs as exp(-0.5 ln(var+eps)) so the Activation engine needs
only one act-function table (no per-element table reloads).
"""

import sys

sys.path.insert(0, "/opt/trn_rl_repo")

from contextlib import ExitStack

import numpy as np

import concourse.bass as bass
import concourse.tile as tile
from concourse import bacc, mybir
from concourse.bass_utils import run_bass_kernel_spmd

B, H, W, C = 32, 32, 32, 512
HW = H * W  # 1024
NCORES = 8
NB = B // NCORES  # 4 batch elements per core
P = 128
GROUPS = 32
EPS = 1e-6
F32 = mybir.dt.float32
BF16 = mybir.dt.bfloat16
FP8 = mybir.dt.float8e4
FP8E5 = mybir.dt.float8e5

CT = C // P  # 4 channel tiles
MT = HW // P  # 8 pixel tiles
WSCALE = 8.0
LN16 = float(np.log(16.0))
DRM = mybir.MatmulPerfMode.DoubleRow


def _pin_act_tables(nc):
    """Force every activation onto one act-function set (ln+exp+copy), so
    the kernel pays a single LoadActFuncSet instead of two per element.

    The stock pass assigns each function its first-containing set (Ln ->
    natural_log set, Exp -> exp set), reloading the table twice per
    element. All functions this kernel uses live in
    `natural_log_exp_and_others`, so presenting that as the only
    non-empty set yields exactly one load.
    """
    import concourse.bacc as bacc_mod
    from concourse.hw_specs import get_activation_tables

    chosen = "natural_log_exp_and_others"

    def patched():
        has_act = any(
            isinstance(i, mybir.InstActivation)
            for b in nc.main_func.blocks
            for i in b.instructions
        )
        if not has_act:
            return
        tables = [
            (k, (v if k == chosen else set()))
            for k, v in get_activation_tables(nc.m.arch).items()
        ]
        bacc_mod._bass_rust.insert_act_table_loads(nc, tables)

    nc.insert_act_table_loads = patched


def build_bass(nb: int = NB, use_bias: bool = False):
    # Bacc (not raw Bass): its finalize() runs generate_event_semaphores,
    # which splits multi-wait instructions to satisfy the 1-wait HW limit.
    nc = bacc.Bacc()
    _pin_act_tables(nc)

    x_in = nc.declare_dram_parameter("xbf16", [nb, HW, C], BF16, isOutput=False)
    gamma_in = nc.declare_dram_parameter("gn_gamma", [C], F32, isOutput=False)
    beta_in = nc.declare_dram_parameter("gn_beta", [C], F32, isOutput=False)
    wq_in = nc.declare_dram_parameter("wq", [C, C], FP8, isOutput=False)
    bq_in = nc.declare_dram_parameter("bq", [C], F32, isOutput=False)
    wk_in = nc.declare_dram_parameter("wk", [C, C], FP8, isOutput=False)
    bk_in = nc.declare_dram_parameter("bk", [C], F32, isOutput=False)
    wv_in = nc.declare_dram_parameter("wv", [C, C], FP8, isOutput=False)
    bv_in = nc.declare_dram_parameter("bv", [C], F32, isOutput=False)
    wo_in = nc.declare_dram_parameter("wo", [C, C], FP8, isOutput=False)
    bo_in = nc.declare_dram_parameter("bo", [C], F32, isOutput=False)
    out_ext = nc.declare_dram_parameter("out", [nb, HW, C], BF16, isOutput=True)

    # Block-diagonal group-averaging matrix: gmat[i, j] = 1/16 iff same group.
    gs = C // GROUPS  # 16 channels per group
    gnp = np.zeros((P, P), dtype=np.float32)
    for g in range(P // gs):
        gnp[g * gs : (g + 1) * gs, g * gs : (g + 1) * gs] = 1.0 / gs
    gmat_dram = nc.inline_tensor(gnp, name="gmat")
    import ml_dtypes

    ident_dram = nc.inline_tensor(
        np.eye(P, dtype=np.float32).astype(ml_dtypes.bfloat16), name="ident"
    )

    inv_sqrt_c = float(C) ** -0.5
    exp_scale = inv_sqrt_c / (WSCALE * WSCALE)

    with tile.TileContext(nc) as tc, ExitStack() as ctx:
        ep = ctx.enter_context

        consts = ep(tc.tile_pool(name="consts", bufs=1))
        wtmp = ep(tc.tile_pool(name="wtmp", bufs=1))
        p_xb = ep(tc.tile_pool(name="p_xb", bufs=3))       # bf16 x; residual
        p_xT = ep(tc.tile_pool(name="p_xT", bufs=3 * CT))
        p_xn = ep(tc.tile_pool(name="p_xn", bufs=2))
        p_st = ep(tc.tile_pool(name="p_st", bufs=12))
        p_q2 = ep(tc.tile_pool(name="p_q2", bufs=2))
        p_pt = ep(tc.tile_pool(name="p_pt", bufs=2))
        p_v = ep(tc.tile_pool(name="p_v", bufs=2))
        p_op = ep(tc.tile_pool(name="p_op", bufs=2))
        p_z = ep(tc.tile_pool(name="p_z", bufs=4))
        p_out = ep(tc.tile_pool(name="p_out", bufs=4))

        # PSUM: one global pool rotating over all 8 banks. Splitting banks
        # per phase caps each phase's matmul-group throughput at
        # bufs/group-latency; a shared FIFO lets any phase use any free
        # bank, roughly doubling PSUM group throughput.
        pq = ep(tc.tile_pool(name="pq", bufs=4, space="PSUM"))

        # ---- small constants first so GroupNorm of elem 0 can start early ----
        gcol = consts.tile([P, CT], F32, name="gamma")
        nc.sync.dma_start(gcol, gamma_in.rearrange("(t p) -> p t", p=P))
        bcol = consts.tile([P, CT], F32, name="beta")
        nc.sync.dma_start(bcol, beta_in.rearrange("(t p) -> p t", p=P))
        gmat_sb = consts.tile([P, P], F32, name="gmat")
        nc.sync.dma_start(gmat_sb, gmat_dram[:, :])
        ident_sb = consts.tile([P, P], BF16, name="ident")
        nc.sync.dma_start(ident_sb, ident_dram[:, :])
        eps_sb = consts.tile([P, 1], F32, name="eps")
        nc.vector.memset(eps_sb, EPS)

        if use_bias:
            bv_col = consts.tile([P, CT], F32, name="bv")
            nc.sync.dma_start(bv_col, bv_in.rearrange("(t p) -> p t", p=P))

        w_sb = {}
        bias_sb = {}
        S = [dict() for _ in range(nb)]

        def dcopy(mode, dst, src):
            if mode == "v":
                nc.vector.tensor_copy(dst, src)
            elif mode == "a":
                nc.scalar.copy(dst, src)
            else:
                nc.gpsimd.tensor_scalar(
                    out=dst, in0=src, scalar1=1.0, scalar2=0.0,
                    op0=mybir.AluOpType.mult, op1=mybir.AluOpType.add,
                )

        def stage_X(i):
            """DMA loads: x^T transposes, residual copy, (weights on i==0)."""
            s = S[i]
            xT = []
            for ct in range(CT):
                tt = p_xT.tile([P, HW], BF16, name="xT")
                nc.sync.dma_start_transpose(tt, x_in[i][:, ct * P : (ct + 1) * P])
                xT.append(tt)
            s["xT"] = xT
            if i == 0:
                for name, wext in (
                    ("q", wq_in), ("k", wk_in), ("v", wv_in), ("o", wo_in)
                ):
                    wb = consts.tile([P, CT, C], FP8, name=f"w_{name}")
                    nc.sync.dma_start(wb, wext.rearrange("(kt p) c -> p kt c", p=P))
                    w_sb[name] = wb
                if use_bias:
                    bq_f32 = wtmp.tile([P, C], F32, name="bqf", tag="wf")
                    nc.sync.dma_start(bq_f32, bq_in[None, :].to_broadcast((P, C)))
                    bq_sb = consts.tile([P, C], BF16, name="bq")
                    nc.vector.tensor_copy(bq_sb, bq_f32)
                    bk_f32 = wtmp.tile([P, C], F32, name="bkf", tag="wf")
                    nc.sync.dma_start(bk_f32, bk_in[None, :].to_broadcast((P, C)))
                    bk_sb = consts.tile([P, C], BF16, name="bk")
                    nc.vector.tensor_copy(bk_sb, bk_f32)
                    bo_sb = consts.tile([P, C], F32, name="bo")
                    nc.sync.dma_start(bo_sb, bo_in[None, :].to_broadcast((P, C)))
                    bias_sb.update(q=bq_sb, k=bk_sb, o=bo_sb)
            xallb = p_xb.tile([P, MT, C], BF16, name="xallb")
            nc.sync.dma_start(xallb, x_in[i].rearrange("(t p) c -> p t c", p=P))
            s["xallb"] = xallb

        def stage_G1(i):
            """GroupNorm statistics (DVE only)."""
            s = S[i]
            mvall = p_st.tile([P, CT, 2], F32, name="mvall", tag="st")
            for ct in range(CT):
                stats = p_st.tile([P, 2, 6], F32, name="bnstats", tag="bs")
                nc.vector.bn_stats(stats[:, 0, :], s["xT"][ct][:, 0:512])
                nc.vector.bn_stats(stats[:, 1, :], s["xT"][ct][:, 512:1024])
                nc.vector.bn_aggr(mvall[:, ct, :], stats)
            # msq = [mean_ch, E[x^2]_ch] per channel tile
            msq = p_st.tile([P, CT, 2], F32, name="msq", tag="st")
            nc.vector.tensor_copy(msq[:, :, 0], mvall[:, :, 0])
            nc.vector.tensor_mul(msq[:, :, 1], mvall[:, :, 0], mvall[:, :, 0])
            nc.vector.tensor_add(msq[:, :, 1], msq[:, :, 1], mvall[:, :, 1])
            s["msq"] = msq

        def stage_G2(i):
            """Group-average (PE), rsqrt via ln/exp (Act), apply (Pool)."""
            s = S[i]
            gps = pq.tile([P, CT, 2], F32, name="gps", tag="ps")
            nc.tensor.matmul(gps, lhsT=gmat_sb, rhs=s["msq"], start=True, stop=True)
            mu_sb = p_st.tile([P, CT], F32, name="mu_sb", tag="st")
            nc.vector.tensor_copy(mu_sb, gps[:, :, 0])
            t2 = p_st.tile([P, CT], F32, name="t2", tag="st")
            nc.vector.tensor_mul(t2, mu_sb, mu_sb)
            varg = p_st.tile([P, CT], F32, name="varg", tag="st")
            nc.vector.tensor_tensor(varg, gps[:, :, 1], t2, mybir.AluOpType.subtract)
            # rsqrt via exp(-0.5 ln(var+eps)): keeps Act on one table set
            lnv = p_st.tile([P, CT], F32, name="lnv", tag="st")
            nc.scalar.activation(
                lnv, varg, mybir.ActivationFunctionType.Ln, bias=eps_sb[:, 0:1]
            )
            sdi = p_st.tile([P, CT], F32, name="sdi", tag="st")
            nc.scalar.activation(
                sdi, lnv, mybir.ActivationFunctionType.Exp, scale=-0.5
            )
            scol = p_st.tile([P, CT], F32, name="scol", tag="st")
            nc.gpsimd.tensor_mul(scol, sdi, gcol)
            sh1 = p_st.tile([P, CT], F32, name="sh1", tag="st")
            nc.gpsimd.tensor_mul(sh1, mu_sb, scol)
            shcol = p_st.tile([P, CT], F32, name="shcol", tag="st")
            nc.gpsimd.tensor_tensor(shcol, bcol, sh1, mybir.AluOpType.subtract)
            xn = p_xn.tile([P, CT, HW], FP8, name="xn")
            for ct in range(CT):
                nc.gpsimd.tensor_scalar(
                    out=xn[:, ct, :],
                    in0=s["xT"][ct],
                    scalar1=scol[:, ct : ct + 1],
                    scalar2=shcol[:, ct : ct + 1],
                    op0=mybir.AluOpType.mult,
                    op1=mybir.AluOpType.add,
                )
            s["xn"] = xn

        QK_ROT = ("a", "v", "a", "v", "a", "v", "v", "v",
                  "a", "v", "a", "v", "a", "v", "v", "v")

        def stage_B(i):
            """q,k projections (Q2/K2 raw-reshape layout) + v projection."""
            s = S[i]
            xn = s["xn"]
            q2sb = p_q2.tile([P, CT, HW], FP8, name="q2", tag="q2")
            k2sb = p_q2.tile([P, CT, HW], FP8, name="k2", tag="k2")
            xnr = xn.rearrange("p c (rt m x) -> p c rt x m", rt=CT, x=2)
            qk_drain = 0
            for rt in range(CT):
                for u in range(2):
                    for big, wname in ((q2sb, "q"), (k2sb, "k")):
                        acc = pq.tile([P, C], F32, name="proj_ps", tag="ps")
                        for j in range(CT // 2):
                            nc.tensor.matmul(
                                acc,
                                lhsT=xnr[:, 2 * j : 2 * j + 2, rt, u, :],
                                rhs=w_sb[wname][:, 2 * j : 2 * j + 2, :],
                                start=(j == 0),
                                stop=(j == CT // 2 - 1),
                                perf_mode=DRM,
                            )
                        dst = big[:, rt, u * 512 : (u + 1) * 512]
                        if use_bias:
                            nc.vector.tensor_add(dst, acc, bias_sb[wname])
                        else:
                            dcopy(QK_ROT[qk_drain], dst, acc)
                        qk_drain += 1
            s["q2"], s["k2"] = q2sb, k2sb

            # v projection (channel-major, even/odd split):
            # v_t[:, 0] = even-pixel halves (+ ones col), v_t[:, 1] = odd
            v_t = p_v.tile([P, 2, CT, 513], FP8, name="v_t")
            nc.vector.memset(v_t[:, :, :, 512:513], WSCALE)
            for ct in range(CT):
                for n in range(2):
                    acc = pq.tile([P, 512], F32, name="proj_ps", tag="ps")
                    for j in range(CT // 2):
                        nc.tensor.matmul(
                            acc,
                            lhsT=w_sb["v"][:, 2 * j : 2 * j + 2, ct * P : (ct + 1) * P],
                            rhs=xn[:, 2 * j : 2 * j + 2, n * 512 : (n + 1) * 512],
                            start=(j == 0),
                            stop=(j == CT // 2 - 1),
                            perf_mode=DRM,
                        )
                    pv = acc.rearrange("p (m two) -> p two m", two=2)
                    dst = v_t[:, :, ct, n * 256 : (n + 1) * 256]
                    if use_bias:
                        nc.vector.tensor_scalar(
                            out=dst, in0=pv, scalar1=bv_col[:, ct : ct + 1],
                            scalar2=None, op0=mybir.AluOpType.add,
                        )
                    else:
                        dcopy("v" if (2 * ct + n) % 2 == 0 else "a", dst, pv)
            s["v_t"] = v_t

        def stage_C(i):
            """S^T = K2^T Q2 (fp8 DR); P^T = exp(S^T/sqrt(c))/16 (Act)."""
            s = S[i]
            PT = p_pt.tile([P, MT, HW], FP8E5, name="pt")
            for bt in range(MT):
                sps = pq.tile([P, 2, 512], F32, name="s_ps", tag="sps", bufs=2)
                for at in range(2):
                    for j in range(CT // 2):
                        nc.tensor.matmul(
                            sps[:, at, :],
                            lhsT=s["k2"][:, 2 * j : 2 * j + 2, bt * P : (bt + 1) * P],
                            rhs=s["q2"][:, 2 * j : 2 * j + 2, at * 512 : (at + 1) * 512],
                            start=(j == 0),
                            stop=(j == CT // 2 - 1),
                            perf_mode=DRM,
                        )
                # PT in e5m2: dynamic range to 57344 so even a 9-sigma
                # logit cannot overflow (e4m3 capped at 240 and one batch
                # element's max logit exp hit ~504)
                nc.scalar.activation(
                    PT[:, bt, :],
                    sps,
                    mybir.ActivationFunctionType.Exp,
                    scale=exp_scale,
                )
            s["PT"] = PT

        FIN_ROT = ("v", "a", "v", "a", "v", "a", "v", "a")

        def stage_D_o(i, am_range):
            """O^T = P @ [V2^T | 8] with 1/Z drain for a block of row tiles.

            ams 0-3 depend only on the at=0 half of the exps, so they are
            emitted before B(i+1); their drains go on Pool which is free
            early in the iteration. Late ams drain on DVE.
            """
            s = S[i]
            PT, v_t = s["PT"], s["v_t"]
            if "opT" not in s:
                s["opT"] = p_op.tile([P, CT, HW], FP8, name="opT")
            opT = s["opT"]
            opv = opT.rearrange("p c (m two) -> p c two m", two=2)
            for am in am_range:
                ops1 = pq.tile([P, 256], F32, name="o_ps1", tag="ps")
                ops2 = pq.tile([P, 257], F32, name="o_ps2", tag="ps")
                for j in range(MT // 2):
                    lhsT = PT[:, 2 * j : 2 * j + 2, am * P : (am + 1) * P]
                    half = 0 if j < 2 else 1
                    cpair = 2 * (j % 2)
                    rhs = v_t[:, half, cpair : cpair + 2, :]
                    nc.tensor.matmul(
                        ops1, lhsT=lhsT, rhs=rhs[:, :, 0:256],
                        start=(j == 0), stop=(j == MT // 2 - 1), perf_mode=DRM,
                    )
                    nc.tensor.matmul(
                        ops2, lhsT=lhsT, rhs=rhs[:, :, 256:513],
                        start=(j == 0), stop=(j == MT // 2 - 1), perf_mode=DRM,
                    )
                zinv = p_z.tile([P, 1], F32, name="zinv")
                nc.vector.reciprocal(zinv, ops2[:, 256:257])
                cht, u = am % CT, am // CT
                dst = opv[:, cht, u, :]
                # odiv drains: multiply by 1/Z; alternate Act (Copy w/ scale)
                # and DVE so neither engine eats the whole O phase
                if am % 2 == 0:
                    nc.scalar.activation(
                        dst[:, 0:256], ops1,
                        mybir.ActivationFunctionType.Copy, scale=zinv[:, 0:1],
                    )
                    nc.scalar.activation(
                        dst[:, 256:512], ops2[:, 0:256],
                        mybir.ActivationFunctionType.Copy, scale=zinv[:, 0:1],
                    )
                else:
                    nc.vector.tensor_scalar_mul(dst[:, 0:256], ops1, zinv)
                    nc.vector.tensor_scalar_mul(
                        dst[:, 256:512], ops2[:, 0:256], zinv
                    )

        def stage_D_fin(i):
            """Final projection + residual + store; residual lands in PSUM
            via an identity matmul for tiles whose drain is a plain copy."""
            s = S[i]
            opT, xallb = s["opT"], s["xallb"]
            for mt in range(MT):
                acc = pq.tile([P, C], F32, name="proj_ps", tag="ps")
                mode = "v" if use_bias else FIN_ROT[mt]
                for j in range(CT // 2):
                    nc.tensor.matmul(
                        acc,
                        lhsT=opT[:, 2 * j : 2 * j + 2, mt * P : (mt + 1) * P],
                        rhs=w_sb["o"][:, 2 * j : 2 * j + 2, :],
                        start=(j == 0),
                        stop=(j == CT // 2 - 1 and mode == "v"),
                        perf_mode=DRM,
                    )
                if mode != "v":
                    nc.tensor.matmul(
                        acc, lhsT=ident_sb, rhs=xallb[:, mt, :],
                        start=False, stop=True,
                    )
                osb = p_out.tile([P, C], BF16, name="osb")
                if use_bias:
                    nc.vector.tensor_add(osb, acc, bias_sb["o"])
                    nc.vector.tensor_add(osb, osb, xallb[:, mt, :])
                elif mode == "v":
                    nc.vector.tensor_add(osb, acc, xallb[:, mt, :])
                else:
                    dcopy(mode, osb, acc)
                nc.sync.dma_start(out_ext[i, mt * P : (mt + 1) * P, :], osb)

        # ---- software-pipelined emission: element i's O/final phase is
        # emitted after element i+1's projections, so the in-order PE has
        # real work while element i's 16 serial exp drains run on Act. ----
        stage_X(0)
        stage_G1(0)
        if nb > 1:
            stage_X(1)
        stage_G2(0)
        if nb > 1:
            stage_G1(1)
        stage_B(0)
        for i in range(nb):
            if i + 2 < nb:
                stage_X(i + 2)
            if i + 1 < nb:
                stage_G2(i + 1)
            stage_C(i)
            stage_D_o(i, range(0, MT // 2))
            if i + 2 < nb:
                stage_G1(i + 2)
            if i + 1 < nb:
                stage_B(i + 1)
            stage_D_o(i, range(MT // 2, MT))
            stage_D_fin(i)

    nc.finalize()
    return nc


_nc_cache = {}


def get_nc(nb: int = NB, use_bias: bool = False):
    key = (nb, use_bias)
    if key not in _nc_cache:
        _nc_cache[key] = build_bass(nb, use_bias)
    return _nc_cache[key]


def host_param(name, value):
    """Host-side preprocessing of a kernel parameter (scaling + dtype cast).

    Kept in one place so test harnesses stay in sync with the kernel's
    on-device dtype expectations.
    """
    import ml_dtypes

    a = np.asarray(value, dtype=np.float32)
    if name in ("wq", "wk", "wv"):
        a = (a * WSCALE).astype(ml_dtypes.float8_e4m3)
    elif name == "wo":
        a = a.astype(ml_dtypes.float8_e4m3)
    elif name in ("bq", "bk", "bv"):
        a = a * WSCALE
    return np.ascontiguousarray(a)


def kernel(x, gn_gamma, gn_beta, wq, bq, wk, bk, wv, bv, wo, bo, **run_kwargs):
    import ml_dtypes

    bf16 = ml_dtypes.bfloat16
    xb = np.ascontiguousarray(
        np.asarray(x, dtype=np.float32).astype(bf16)
    ).reshape(B, HW, C)
    raw = {
        "gn_gamma": gn_gamma, "gn_beta": gn_beta,
        "wq": wq, "bq": bq, "wk": wk, "bk": bk,
        "wv": wv, "bv": bv, "wo": wo, "bo": bo,
    }
    use_bias = any(
        np.any(np.asarray(raw[k], dtype=np.float32) != 0.0)
        for k in ("bq", "bk", "bv", "bo")
    )
    params = {k: host_param(k, v) for k, v in raw.items()}
    nc = get_nc(NB, use_bias)
    in_maps = [
        {"xbf16": xb[i * NB : (i + 1) * NB], **params} for i in range(NCORES)
    ]
    res = run_bass_kernel_spmd(nc, in_maps, core_ids=list(range(NCORES)), **run_kwargs)
    global last_results
    last_results = res
    out = np.concatenate([res.results[i]["out"] for i in range(NCORES)], axis=0)
    return out.reshape(B, H, W, C).astype(np.float32)


last_results = None


if __name__ == "__main__":
    nc = build_bass(NB)
    print("build + compile OK")
